# revision 1
# baseline (speedup 1.0000x reference)
"""Trainium2 Bass kernel for a dense transformer block (B=2, T=2048, C=1024,
H=16 heads, HS=64, FF=4096, fp32), SPMD across 8 NeuronCores.

Sharding strategy
-----------------
- LayerNorms + FFN + proj: sequence-parallel. Core c owns 512 tokens
  (rows 512c..512c+511 of the flattened [4096, 1024] activation).
- Attention: head-parallel. Core c owns heads 2c and 2c+1 over all tokens.
- Collectives (bf16 wire, ~1MB/rank):
  1. AllGather of h^T (LN1 output, transposed) so every core can compute
     Q/K/V for its heads over all 4096 tokens.
  2. AllToAll of att^T to re-shard from head-sharded to token-sharded for
     the output projection.
  The final output needs no collective: each core returns its token chunk
  and the host concatenates.

Numerics: matmul operands in bf16 (fp32 PSUM accumulate); LayerNorm
stats/apply, softmax exp, normalization, and residuals in fp32.
LayerNorm scale/bias and the per-head attention scale p^-0.5 are folded
into the weight matrices on the host, so on-device LN is just
(x - mean) * rstd. The K-projection bias is dropped entirely: softmax
over keys is invariant to a per-query constant score offset.

Layout: activations feeding matmul contractions over channels are kept
transposed ([channel, token]); LN outputs are transposed on the PE
(128x128 tiles). Softmax runs in the S^T = [key, query] orientation;
the denominator comes free as a ones-column appended to V, and its
reciprocal is broadcast across partitions with a rank-1 PE matmul.
"""

import os
import numpy as np

B, T, C = 2, 2048, 1024
H, HS = 16, 64
FF = 4 * C
EPS = 1e-5
NCORE = 8
TOK = B * T            # 4096 flattened tokens
CHUNK = TOK // NCORE   # 512 tokens per core
P = 128
NTT = CHUNK // P       # 4 token tiles of 128 per core
NG = C // P            # 8 channel chunks
NF = FF // P           # 32 ff slices
LH = 2                 # local heads per core

_BUILT = None


def _build():
    import concourse.bass as bass
    import concourse.tile as tile
    from concourse import bacc, mybir
    from concourse.masks import make_identity
    from contextlib import ExitStack

    f32 = mybir.dt.float32
    bf16 = mybir.dt.bfloat16
    Alu = mybir.AluOpType
    Act = mybir.ActivationFunctionType

    nc = bacc.Bacc("TRN2", target_bir_lowering=False, debug=False,
                   num_devices=NCORE)

    xc = nc.dram_tensor("xc", [CHUNK, C], f32, kind="ExternalInput").ap()
    wqkv = nc.dram_tensor("wqkv", [C, 3 * P], bf16, kind="ExternalInput").ap()
    bqk = nc.dram_tensor("bqk", [P, LH], f32, kind="ExternalInput").ap()
    bv = nc.dram_tensor("bv", [P, 1], f32, kind="ExternalInput").ap()
    wproj = nc.dram_tensor("wproj", [C, C], bf16, kind="ExternalInput").ap()
    w1 = nc.dram_tensor("w1", [C, FF], bf16, kind="ExternalInput").ap()
    bff1 = nc.dram_tensor("bff1", [P, NF], f32, kind="ExternalInput").ap()
    w2 = nc.dram_tensor("w2", [FF, C], bf16, kind="ExternalInput").ap()
    out = nc.dram_tensor("out", [CHUNK, C], f32, kind="ExternalOutput").ap()
    DEBUG = bool(int(os.environ.get("BASSK_DEBUG", "0")))
    if DEBUG:
        dbg_h = nc.dram_tensor("dbg_h", [C, CHUNK], bf16,
                               kind="ExternalOutput").ap()
        dbg_qkT = nc.dram_tensor("dbg_qkT", [P, LH, TOK], bf16,
                                 kind="ExternalOutput").ap()
        dbg_v = nc.dram_tensor("dbg_v", [P, TOK // P, 132], bf16,
                               kind="ExternalOutput").ap()
        dbg_att = nc.dram_tensor("dbg_att", [NCORE, P, CHUNK], bf16,
                                 kind="ExternalOutput").ap()
        dbg_xmid = nc.dram_tensor("dbg_xmid", [P, NTT, C], f32,
                                  kind="ExternalOutput").ap()

    # collective bounce buffers (internal DRAM; AllGather output is Shared)
    h_bounce = nc.dram_tensor("h_bounce", [C, CHUNK], bf16)
    hT_all = nc.dram_tensor("hT_all", [NCORE * C, CHUNK], bf16,
                            addr_space="Shared")
    attT_bounce = [nc.dram_tensor(f"attT_bounce{i}", [NCORE, 64, CHUNK], bf16)
                   for i in range(LH)]
    attT_recv = [nc.dram_tensor(f"attT_recv{i}", [NCORE, 64, CHUNK], bf16)
                 for i in range(LH)]
    groups = [list(range(NCORE))]

    with tile.TileContext(nc) as tc, ExitStack() as top:
        const = top.enter_context(tc.tile_pool(name="const", bufs=1))
        persist = top.enter_context(tc.tile_pool(name="persist", bufs=1))
        ps = top.enter_context(tc.tile_pool(name="ps", bufs=4, space="PSUM"))
        ps2 = top.enter_context(tc.tile_pool(name="ps2", bufs=2, space="PSUM"))

        # round-robin DMA issue across engine queues to spread bandwidth
        dma_engines = [nc.sync, nc.scalar]

        def dma(i, **kw):
            dma_engines[i % len(dma_engines)].dma_start(**kw)

        ident = const.tile([P, P], bf16)
        make_identity(nc, ident)
        ones1 = const.tile([1, 64], bf16)
        nc.vector.memset(ones1, 1.0)
        eps_sb = const.tile([P, 1], f32)
        nc.vector.memset(eps_sb, EPS)

        xc_sb = persist.tile([P, NTT, C], f32)
        xmid_sb = persist.tile([P, NTT, C], f32)
        wqkv_sb = persist.tile([P, NG, 3 * P], bf16)
        bqk_sb = persist.tile([P, LH], f32)
        bv_sb = persist.tile([P, 1], f32)
        bff1_sb = persist.tile([P, NF], f32)
        w1p = top.enter_context(tc.tile_pool(name="w1p", bufs=1))
        w1_sb = w1p.tile([P, NG, FF], bf16)   # prefetched whole

        for jt in range(NTT):
            nc.sync.dma_start(out=xc_sb[:, jt, :],
                              in_=xc[P * jt:P * (jt + 1), :])
        nc.sync.dma_start(out=wqkv_sb,
                          in_=wqkv.rearrange("(g p) m -> p g m", p=P))
        nc.sync.dma_start(out=bqk_sb, in_=bqk)
        nc.sync.dma_start(out=bv_sb, in_=bv)
        nc.sync.dma_start(out=bff1_sb, in_=bff1)
        # w1 prefetch rides the otherwise-idle gpsimd SWDGE queue
        for g in range(NG):
            nc.gpsimd.dma_start(out=w1_sb[:, g, :],
                                in_=w1[P * g:P * (g + 1), :])

        def layernorm_tile(pool, src_ap, out_dt):
            """src_ap: [P, C] fp32 in SBUF -> normalized [P, C] tile."""
            stats = pool.tile([P, 2, 6], f32, tag="ln_stats")
            nc.vector.bn_stats(out=stats[:, 0, :], in_=src_ap[:, 0:512])
            nc.vector.bn_stats(out=stats[:, 1, :], in_=src_ap[:, 512:1024])
            mv = pool.tile([P, 2], f32, tag="ln_mv")
            nc.vector.bn_aggr(out=mv, in_=stats)
            rstd = pool.tile([P, 1], f32, tag="ln_rstd")
            nc.scalar.activation(rstd, mv[:, 1:2], Act.Sqrt, bias=eps_sb)
            nc.vector.reciprocal(rstd, rstd)
            negmr = pool.tile([P, 1], f32, tag="ln_negmr")
            nc.vector.tensor_scalar(negmr, mv[:, 0:1], rstd, -1.0,
                                    Alu.mult, Alu.mult)
            hn = pool.tile([P, C], out_dt, tag="ln_out")
            nc.scalar.activation(hn, src_ap, Act.Identity,
                                 bias=negmr, scale=rstd)
            return hn

        # ---------------- Stage A: LN1 + transpose + AllGather --------------
        with ExitStack() as sa:
            lnp = sa.enter_context(tc.tile_pool(name="lnp", bufs=3))
            for jt in range(NTT):
                hn = layernorm_tile(lnp, xc_sb[:, jt, :], bf16)
                for g in range(NG):
                    tp = ps.tile([P, P], bf16, tag="bank")
                    nc.tensor.transpose(tp, hn[:, P * g:P * (g + 1)], ident)
                    hb = lnp.tile([P, P], bf16, tag="htout")
                    nc.vector.tensor_copy(hb, tp)
                    nc.sync.dma_start(
                        out=h_bounce[P * g:P * (g + 1),
                                     P * jt:P * (jt + 1)], in_=hb)
                    if DEBUG:
                        nc.sync.dma_start(
                            out=dbg_h[P * g:P * (g + 1),
                                      P * jt:P * (jt + 1)], in_=hb)
            nc.gpsimd.collective_compute(
                "AllGather", Alu.bypass, replica_groups=groups,
                ins=[h_bounce[:, :]], outs=[hT_all[:, :]])
            tc.no_sync_barrier()

        # ---------------- Stage B: QKV + attention --------------------------
        with ExitStack() as sb:
            qkp = sb.enter_context(tc.tile_pool(name="qkp", bufs=1))
            htp = sb.enter_context(tc.tile_pool(name="htp", bufs=2))
            vtp = sb.enter_context(tc.tile_pool(name="vtp", bufs=2))

            qkT = qkp.tile([P, LH, TOK], bf16)  # rows 0:64 Q^T, 64:128 K^T
            # K^T copied down to base partition 0 (matmul needs equal
            # base_partition on both operands)
            kT = qkp.tile([64, LH, TOK], bf16)
            Vsb = qkp.tile([P, TOK // P, 132], bf16)
            nc.vector.memset(Vsb[:, :, 64:65], 1.0)
            nc.vector.memset(Vsb[:, :, 130:131], 1.0)

            for rr in range(NCORE):
                ht = htp.tile([P, NG, CHUNK], bf16, tag="ht")
                nc.sync.dma_start(out=ht,
                    in_=hT_all[C * rr:C * (rr + 1), :].rearrange(
                        "(g p) m -> p g m", p=P))
                for hp in range(LH):
                    psA = ps.tile([P, CHUNK], f32, tag="bank")
                    for g in range(NG):
                        nc.tensor.matmul(
                            psA, wqkv_sb[:, g, P * hp:P * (hp + 1)],
                            ht[:, g, :], start=(g == 0), stop=(g == NG - 1))
                    nc.vector.tensor_scalar_add(
                        qkT[:, hp, CHUNK * rr:CHUNK * (rr + 1)], psA,
                        bqk_sb[:, hp:hp + 1])
                    nc.sync.dma_start(
                        out=kT[:, hp, CHUNK * rr:CHUNK * (rr + 1)],
                        in_=qkT[64:128, hp, CHUNK * rr:CHUNK * (rr + 1)])
                psV = ps.tile([P, CHUNK], f32, tag="bank")
                for g in range(NG):
                    nc.tensor.matmul(psV, wqkv_sb[:, g, 2 * P:3 * P],
                                     ht[:, g, :], start=(g == 0),
                                     stop=(g == NG - 1))
                vt = vtp.tile([P, CHUNK], bf16, tag="vt")
                nc.vector.tensor_scalar_add(vt, psV, bv_sb)
                for tt in range(NTT):
                    tpv = ps.tile([P, P], bf16, tag="bank")
                    nc.tensor.transpose(tpv, vt[:, P * tt:P * (tt + 1)], ident)
                    vdst = Vsb[:, NTT * rr + tt, :].rearrange(
                        "p (a b) -> p a b", a=2)[:, :, 0:64]
                    vsrc = tpv.rearrange("p (a b) -> p a b", a=2)
                    nc.vector.tensor_copy(vdst, vsrc)

            # attention: per local head hp, batch b, query tile jq (512 wide)
            atp = sb.enter_context(tc.tile_pool(name="atp", bufs=4))
            ate = sb.enter_context(tc.tile_pool(name="ate", bufs=2))
            for hp in range(LH):
                for b in range(B):
                    base_t = T * b
                    for jq in range(4):
                        q0 = base_t + 512 * jq
                        nk = 4 * (jq + 1)
                        psPV = ps.tile([65, 512], f32, tag="bank")
                        for ike in range(0, nk, 2):
                            psS2 = ps2.tile([P, 1024], f32, tag="bank2")
                            for dd in range(2):
                                ik = ike + dd
                                k0 = base_t + P * ik
                                nc.tensor.matmul(
                                    psS2[:, 512 * dd:512 * (dd + 1)],
                                    kT[:, hp, k0:k0 + P],
                                    qkT[0:64, hp, q0:q0 + 512],
                                    start=True, stop=True)
                            pt = atp.tile([P, 1024], bf16, tag="pt")
                            nc.scalar.activation(pt, psS2, Act.Exp)
                            for dd in range(2):
                                ik = ike + dd
                                ph = pt[:, 512 * dd:512 * (dd + 1)]
                                diag = 512 * jq - P * ik
                                if diag < P:  # diagonal block: causal mask
                                    nc.gpsimd.affine_select(
                                        out=ph, in_=ph, pattern=[[1, 512]],
                                        compare_op=Alu.is_ge, fill=0.0,
                                        base=diag, channel_multiplier=-1)
                                nc.tensor.matmul(
                                    psPV, Vsb[:, (base_t // P) + ik,
                                              66 * hp:66 * hp + 65],
                                    ph, start=(ik == 0), stop=(ik == nk - 1))
                        rs = ate.tile([1, 512], f32, tag="rs")
                        nc.vector.tensor_copy(rs, psPV[64:65, :])
                        rec_f = ate.tile([1, 512], f32, tag="rec_f")
                        nc.vector.reciprocal_approx_fast(rec_f, rs)
                        rec = ate.tile([1, 512], bf16, tag="rec")
                        nc.vector.tensor_copy(rec, rec_f)
                        psBC = ps.tile([64, 512], f32, tag="bank")
                        nc.tensor.matmul(psBC, ones1, rec,
                                         start=True, stop=True)
                        bc = ate.tile([64, 512], f32, tag="bc")
                        nc.vector.tensor_copy(bc, psBC)
                        att = ate.tile([64, 512], bf16, tag="attout")
                        nc.vector.tensor_mul(att, psPV[0:64, :], bc)
                        nc.sync.dma_start(
                            out=attT_bounce[hp][4 * b + jq, :, :],
                            in_=att)
                if b == B - 1:
                    nc.gpsimd.collective_compute(
                        "AllToAll", Alu.bypass, replica_groups=groups,
                        ins=[attT_bounce[hp][:, :, :]],
                        outs=[attT_recv[hp][:, :, :]])
            if DEBUG:
                nc.sync.dma_start(out=dbg_qkT, in_=qkT)
                nc.sync.dma_start(out=dbg_v, in_=Vsb)
            tc.no_sync_barrier()

        # ---------------- Stage C: proj + residual --------------------------
        with ExitStack() as sc:
            prp = sc.enter_context(tc.tile_pool(name="prp", bufs=8))
            ats = []
            for g in range(NG):
                at = prp.tile([P, CHUNK], bf16, tag="at", name=f"at{g}")
                nc.sync.dma_start(out=at[0:64, :], in_=attT_recv[0][g, :, :])
                nc.sync.dma_start(out=at[64:128, :], in_=attT_recv[1][g, :, :])
                ats.append(at)
            wpp = sc.enter_context(tc.tile_pool(name="wpp", bufs=2))
            for n in range(2):
                wp = wpp.tile([P, NG, 512], bf16, tag="wp")
                dma(n, out=wp,
                    in_=wproj[:, 512 * n:512 * (n + 1)].rearrange(
                        "(g p) m -> p g m", p=P))
                for jp in range(2):  # token-tile pairs; 2 live accumulators
                    psj = [ps2.tile([P, 512], f32, tag="bank2",
                                    name=f"psj{n}_{jp}_{jj}")
                           for jj in range(2)]
                    for g in range(NG):
                        for jj in range(2):
                            jt = 2 * jp + jj
                            nc.tensor.matmul(
                                psj[jj], ats[g][:, P * jt:P * (jt + 1)],
                                wp[:, g, :],
                                start=(g == 0), stop=(g == NG - 1))
                    for jj in range(2):
                        jt = 2 * jp + jj
                        nc.vector.tensor_add(
                            xmid_sb[:, jt, 512 * n:512 * (n + 1)], psj[jj],
                            xc_sb[:, jt, 512 * n:512 * (n + 1)])

        if DEBUG:
            nc.sync.dma_start(out=dbg_xmid, in_=xmid_sb)

        # ---------------- Stage D: LN2 + FFN + residual ---------------------
        with ExitStack() as sd:
            ffp = sd.enter_context(tc.tile_pool(name="ffp", bufs=1))
            lnp2 = sd.enter_context(tc.tile_pool(name="lnp2", bufs=3))
            w2p = sd.enter_context(tc.tile_pool(name="w2p", bufs=1))
            outp = sd.enter_context(tc.tile_pool(name="outp", bufs=3))

            h2T = ffp.tile([P, NG, CHUNK], bf16)
            ff1T = ffp.tile([P, NF, CHUNK], bf16)

            for jt in range(NTT):
                hn2 = layernorm_tile(lnp2, xmid_sb[:, jt, :], bf16)
                for g in range(NG):
                    tp = ps.tile([P, P], bf16, tag="bank")
                    nc.tensor.transpose(tp, hn2[:, P * g:P * (g + 1)], ident)
                    nc.vector.tensor_copy(
                        h2T[:, g, P * jt:P * (jt + 1)], tp)

            for f in range(NF):
                psF = ps2.tile([P, CHUNK], f32, tag="bank2")
                for g in range(NG):
                    nc.tensor.matmul(psF, w1_sb[:, g, P * f:P * (f + 1)],
                                     h2T[:, g, :],
                                     start=(g == 0), stop=(g == NG - 1))
                nc.scalar.activation(ff1T[:, f, :], psF, Act.Relu,
                                     bias=bff1_sb[:, f:f + 1])

            for n in range(2):
                psj = [ps.tile([P, 512], f32, tag="bank", name=f"psk{n}_{jt}")
                       for jt in range(NTT)]
                w2n = w2p.tile([P, NF, 512], bf16, tag="w2n")
                for a in range(4):
                    dma(a + n, out=w2n[:, 8 * a:8 * (a + 1), :],
                        in_=w2[1024 * a:1024 * (a + 1),
                               512 * n:512 * (n + 1)].rearrange(
                                   "(q p) m -> p q m", p=P))
                for q in range(NF):
                    for jt in range(NTT):
                        nc.tensor.matmul(
                            psj[jt], ff1T[:, q, P * jt:P * (jt + 1)],
                            w2n[:, q, :], start=(q == 0), stop=(q == NF - 1))
                for jt in range(NTT):
                    ot = outp.tile([P, 512], f32, tag="outt")
                    nc.vector.tensor_add(ot, psj[jt],
                                         xmid_sb[:, jt, 512 * n:512 * (n + 1)])
                    nc.sync.dma_start(
                        out=out[P * jt:P * (jt + 1), 512 * n:512 * (n + 1)],
                        in_=ot)

    nc.compile()
    return nc


def _prepare_inputs(x, Wq, Wk, Wv, p, Wproj, W1, W2,
                    ln1_w, ln1_b, ln2_w, ln2_b):
    import ml_dtypes
    f = np.float32
    bf = ml_dtypes.bfloat16
    x = np.asarray(x, f).reshape(TOK, C)
    Wq, Wk, Wv = (np.asarray(a, f) for a in (Wq, Wk, Wv))
    p = np.asarray(p, f)
    Wproj = np.asarray(Wproj, f)
    W1, W2 = np.asarray(W1, f), np.asarray(W2, f)
    ln1_w, ln1_b = np.asarray(ln1_w, f), np.asarray(ln1_b, f)
    ln2_w, ln2_b = np.asarray(ln2_w, f), np.asarray(ln2_b, f)

    s = (p.astype(np.float64) ** -0.5).astype(f)

    w1_f = np.ascontiguousarray((ln2_w[:, None] * W1).astype(bf))
    bff1 = ln2_b @ W1
    bff1 = np.ascontiguousarray(bff1.reshape(NF, P).T.astype(f))
    w2_bf = np.ascontiguousarray(W2.astype(bf))
    wproj_bf = np.ascontiguousarray(Wproj.astype(bf))

    in_maps = []
    for c in range(NCORE):
        h0, h1 = 2 * c, 2 * c + 1
        blocks = []
        bqk_cols = []
        for h in (h0, h1):
            wq_f = ln1_w[:, None] * Wq[h] * s[h]
            wk_f = ln1_w[:, None] * Wk[h]
            blocks.append(np.concatenate([wq_f, wk_f], axis=1))
            # K bias intentionally zero: softmax is invariant to it
            bqk_cols.append(np.concatenate(
                [s[h] * (ln1_b @ Wq[h]), np.zeros(HS, f)]))
        wv_f = np.concatenate(
            [ln1_w[:, None] * Wv[h0], ln1_w[:, None] * Wv[h1]], axis=1)
        blocks.append(wv_f)
        wqkv_c = np.ascontiguousarray(
            np.concatenate(blocks, axis=1).astype(bf))
        bqk_c = np.ascontiguousarray(np.stack(bqk_cols, axis=1).astype(f))
        bv_c = np.ascontiguousarray(np.concatenate(
            [ln1_b @ Wv[h0], ln1_b @ Wv[h1]])[:, None].astype(f))
        in_maps.append({
            "xc": np.ascontiguousarray(x[CHUNK * c:CHUNK * (c + 1)]),
            "wqkv": wqkv_c,
            "bqk": bqk_c,
            "bv": bv_c,
            "wproj": wproj_bf,
            "w1": w1_f,
            "bff1": bff1,
            "w2": w2_bf,
        })
    return in_maps


def kernel(**inputs):
    global _BUILT
    from concourse.bass_utils import run_bass_kernel_spmd

    if _BUILT is None:
        _BUILT = _build()
    in_maps = _prepare_inputs(**inputs)
    trace = bool(int(os.environ.get("BASSK_TRACE", "0")))
    res = run_bass_kernel_spmd(_BUILT, in_maps, list(range(NCORE)),
                               trace=trace)
    if trace:
        kernel.last_exec_time_ns = res.exec_time_ns
        kernel.last_res = res
    out = np.concatenate([res.results[c]["out"] for c in range(NCORE)], axis=0)
    return out.reshape(B, T, C).astype(np.float32)



# revision 8
# speedup vs baseline: 1.2644x; 1.2644x over previous
"""Trainium2 Bass kernel for a dense transformer block (B=2, T=2048, C=1024,
H=16 heads, HS=64, FF=4096, fp32), SPMD across 8 NeuronCores.

Sharding strategy (v2 — AllGather-free)
---------------------------------------
Core c owns 512 tokens (rows 512c..512c+511 of the flattened [4096, 1024]
activation) for LayerNorms, QKV projection, proj and FFN; attention is
head-parallel (core c owns heads 2c, 2c+1 over all tokens).

Instead of AllGather-ing LN1 output (8 MB out, ~70us RDH) and computing
QKV redundantly per head, each core projects Q/K/V for ALL heads over its
OWN 512 tokens (same FLOPs), then three 1 MB AllToAlls reshard Q^T, K^T
and V from token-sharded to head-sharded. The A2As pipeline behind the
QKV matmuls. Attention output is resharded back with two per-head 0.5 MB
AllToAlls (the first overlaps the second head's compute), then proj + FFN
run token-sharded with no further communication.

Numerics: matmul operands bf16 (fp32 PSUM accumulate); LayerNorm stats,
softmax exp and normalization in fp32. LN scale/bias and the per-head
attention scale p^-0.5 are folded into the weights on the host; the
K-projection bias is dropped (softmax invariance). All weights are
host-relaid to [128-partition, ...] contiguous layout so every weight DMA
is a single large contiguous transfer.

Layout: Q^T/K^T arrive per head-pair as [128 = 2x64 dims, tokens]; for
head hp both S-matmul operands sit at base partition 64*hp, so no K
re-basing copy is needed. Softmax runs in S^T = [key, query] orientation;
the denominator comes free as a ones-column appended to V, its reciprocal
broadcast across partitions with a rank-1 PE matmul.
"""

import os
import numpy as np

B, T, C = 2, 2048, 1024
H, HS = 16, 64
FF = 4 * C
EPS = 1e-5
NCORE = 8
TOK = B * T            # 4096 flattened tokens
CHUNK = TOK // NCORE   # 512 tokens per core
P = 128
NTT = CHUNK // P       # 4 token tiles of 128 per core
NG = C // P            # 8 channel chunks
NF = FF // P           # 32 ff slices
LH = 2                 # local heads per core

_BUILT = None


def _build():
    import concourse.bass as bass
    import concourse.tile as tile
    from concourse import bacc, mybir
    from concourse.masks import make_identity
    from contextlib import ExitStack

    f32 = mybir.dt.float32
    bf16 = mybir.dt.bfloat16
    Alu = mybir.AluOpType
    Act = mybir.ActivationFunctionType

    nc = bacc.Bacc("TRN2", target_bir_lowering=False, debug=False,
                   num_devices=NCORE)

    xc = nc.dram_tensor("xc", [CHUNK, C], f32, kind="ExternalInput").ap()
    wk = nc.dram_tensor("wk", [P, NG, C], bf16, kind="ExternalInput").ap()
    wq = nc.dram_tensor("wq", [P, NG, C], bf16, kind="ExternalInput").ap()
    wv = nc.dram_tensor("wv", [P, NG, C], bf16, kind="ExternalInput").ap()
    bq = nc.dram_tensor("bq", [P, NCORE], f32, kind="ExternalInput").ap()
    bv = nc.dram_tensor("bv", [P, NCORE], f32, kind="ExternalInput").ap()
    wproj = nc.dram_tensor("wproj", [P, NG, C], bf16,
                           kind="ExternalInput").ap()
    w1 = nc.dram_tensor("w1", [P, NG, FF], bf16, kind="ExternalInput").ap()
    bff1 = nc.dram_tensor("bff1", [P, NF], f32, kind="ExternalInput").ap()
    w2a = nc.dram_tensor("w2a", [P, NF, 512], bf16, kind="ExternalInput").ap()
    w2b = nc.dram_tensor("w2b", [P, NF, 512], bf16, kind="ExternalInput").ap()
    out = nc.dram_tensor("out", [CHUNK, C], f32, kind="ExternalOutput").ap()

    # collective buffers (internal DRAM; outputs Shared)
    a2a_k_in = nc.dram_tensor("a2a_k_in", [NCORE, P, CHUNK], bf16)
    a2a_k_out = nc.dram_tensor("a2a_k_out", [NCORE, P, CHUNK], bf16)
    a2a_q_in = nc.dram_tensor("a2a_q_in", [NCORE, P, CHUNK], bf16)
    a2a_q_out = nc.dram_tensor("a2a_q_out", [NCORE, P, CHUNK], bf16)
    a2a_v_in = nc.dram_tensor("a2a_v_in", [NCORE, P, NTT, P], bf16)
    a2a_v_out = nc.dram_tensor("a2a_v_out", [NCORE, P, NTT, P], bf16)
    att_in = [nc.dram_tensor(f"att_in{i}", [NCORE, 64, CHUNK], bf16)
              for i in range(LH)]
    att_out = [nc.dram_tensor(f"att_out{i}", [NCORE, 64, CHUNK], bf16)
               for i in range(LH)]
    groups = [list(range(NCORE))]

    with tile.TileContext(nc) as tc, ExitStack() as top:
        const = top.enter_context(tc.tile_pool(name="const", bufs=1))
        persist = top.enter_context(tc.tile_pool(name="persist", bufs=1))
        attd = top.enter_context(tc.tile_pool(name="attd", bufs=1))
        ps = top.enter_context(tc.tile_pool(name="ps", bufs=4, space="PSUM"))
        ps2 = top.enter_context(tc.tile_pool(name="ps2", bufs=2, space="PSUM"))

        ident = const.tile([P, P], bf16)
        make_identity(nc, ident)
        ones1 = const.tile([1, 64], bf16)
        nc.vector.memset(ones1, 1.0)
        eps_sb = const.tile([P, 1], f32)
        nc.vector.memset(eps_sb, EPS)
        # causal masks for the 4 diagonal-block offsets: keep q >= p + 128*d
        masks = []
        for dmask in range(4):
            mk = const.tile([P, 512], bf16, tag=f"mk{dmask}")
            nc.vector.memset(mk, 1.0)
            nc.gpsimd.affine_select(
                out=mk, in_=mk, pattern=[[1, 512]],
                compare_op=Alu.is_ge, fill=0.0,
                base=-P * dmask, channel_multiplier=-1)
            masks.append(mk)

        xc_sb = persist.tile([P, NTT, C], f32)
        xmid_sb = persist.tile([P, NTT, C], f32)
        hT = persist.tile([P, NG, CHUNK], bf16)
        bq_sb = persist.tile([P, NCORE], f32)
        bv_sb = persist.tile([P, NCORE], f32)
        bff1_sb = persist.tile([P, NF], f32)

        # attention data: Q^T/K^T per head-pair [2x64 dims, all tokens],
        # V token-major with a ones column per head for the softmax denom
        qT = attd.tile([P, NCORE, CHUNK], bf16)
        kT = attd.tile([P, NCORE, CHUNK], bf16)
        Vsb = attd.tile([P, TOK // P, 132], bf16)

        # input DMAs: activations on the sync HWDGE ring
        for jt in range(NTT):
            nc.sync.dma_start(out=xc_sb[:, jt, :],
                              in_=xc[P * jt:P * (jt + 1), :])
        nc.sync.dma_start(out=bq_sb, in_=bq)
        nc.sync.dma_start(out=bv_sb, in_=bv)
        nc.sync.dma_start(out=bff1_sb, in_=bff1)

        def layernorm_tile(pool, src_ap, out_dt):
            """src_ap: [P, C] fp32 in SBUF -> normalized [P, C] tile."""
            stats = pool.tile([P, 2, 6], f32, tag="ln_stats")
            nc.vector.bn_stats(out=stats[:, 0, :], in_=src_ap[:, 0:512])
            nc.vector.bn_stats(out=stats[:, 1, :], in_=src_ap[:, 512:1024])
            mv = pool.tile([P, 2], f32, tag="ln_mv")
            nc.vector.bn_aggr(out=mv, in_=stats)
            rstd = pool.tile([P, 1], f32, tag="ln_rstd")
            nc.scalar.activation(rstd, mv[:, 1:2], Act.Sqrt, bias=eps_sb)
            nc.vector.reciprocal(rstd, rstd)
            negmr = pool.tile([P, 1], f32, tag="ln_negmr")
            nc.vector.tensor_scalar(negmr, mv[:, 0:1], rstd, -1.0,
                                    Alu.mult, Alu.mult)
            hn = pool.tile([P, C], out_dt, tag="ln_out")
            nc.scalar.activation(hn, src_ap, Act.Identity,
                                 bias=negmr, scale=rstd)
            return hn

        # ------------- Stage A: LN1 + transpose (local chunk only) ----------
        # ------------- Stage B: QKV for all heads + 3 AllToAlls -------------
        with ExitStack() as sa:
            wqkvp = sa.enter_context(tc.tile_pool(name="wqkvp", bufs=1))
            lnp = sa.enter_context(tc.tile_pool(name="lnp", bufs=3))
            qkvb = sa.enter_context(tc.tile_pool(name="qkvb", bufs=3))

            wk_sb = wqkvp.tile([P, NG, C], bf16)
            wq_sb = wqkvp.tile([P, NG, C], bf16)
            wv_sb = wqkvp.tile([P, NG, C], bf16)
            nc.scalar.dma_start(out=wk_sb, in_=wk)
            nc.scalar.dma_start(out=wq_sb, in_=wq)
            nc.scalar.dma_start(out=wv_sb, in_=wv)

            for jt in range(NTT):
                hn = layernorm_tile(lnp, xc_sb[:, jt, :], bf16)
                for g in range(NG):
                    tp = ps.tile([P, P], bf16, tag="bank")
                    nc.tensor.transpose(tp, hn[:, P * g:P * (g + 1)], ident)
                    nc.vector.tensor_copy(hT[:, g, P * jt:P * (jt + 1)], tp)

            # K projection for every destination core, then A2A
            for j in range(NCORE):
                psK = ps.tile([P, CHUNK], f32, tag="bank")
                for g in range(NG):
                    nc.tensor.matmul(psK, wk_sb[:, g, P * j:P * (j + 1)],
                                     hT[:, g, :], start=(g == 0),
                                     stop=(g == NG - 1))
                kb = qkvb.tile([P, CHUNK], bf16, tag="kb")
                nc.vector.tensor_copy(kb, psK)
                nc.sync.dma_start(out=a2a_k_in[j], in_=kb)
            nc.gpsimd.collective_compute(
                "AllToAll", Alu.bypass, replica_groups=groups,
                ins=[a2a_k_in[:, :, :]], outs=[a2a_k_out[:, :, :]])

            # Q projection (+ bias), then A2A
            for j in range(NCORE):
                psQ = ps.tile([P, CHUNK], f32, tag="bank")
                for g in range(NG):
                    nc.tensor.matmul(psQ, wq_sb[:, g, P * j:P * (j + 1)],
                                     hT[:, g, :], start=(g == 0),
                                     stop=(g == NG - 1))
                qb = qkvb.tile([P, CHUNK], bf16, tag="qb")
                nc.vector.tensor_scalar_add(qb, psQ, bq_sb[:, j:j + 1])
                nc.sync.dma_start(out=a2a_q_in[j], in_=qb)
            nc.gpsimd.collective_compute(
                "AllToAll", Alu.bypass, replica_groups=groups,
                ins=[a2a_q_in[:, :, :]], outs=[a2a_q_out[:, :, :]])

            # V projection (+ bias) + transpose to token-major, then A2A
            for j in range(NCORE):
                psV = ps.tile([P, CHUNK], f32, tag="bank")
                for g in range(NG):
                    nc.tensor.matmul(psV, wv_sb[:, g, P * j:P * (j + 1)],
                                     hT[:, g, :], start=(g == 0),
                                     stop=(g == NG - 1))
                vt = qkvb.tile([P, CHUNK], bf16, tag="vt")
                nc.vector.tensor_scalar_add(vt, psV, bv_sb[:, j:j + 1])
                vloc = qkvb.tile([P, NTT, P], bf16, tag="vloc")
                for tt in range(NTT):
                    tpv = ps.tile([P, P], bf16, tag="bank")
                    nc.tensor.transpose(tpv, vt[:, P * tt:P * (tt + 1)], ident)
                    nc.vector.tensor_copy(vloc[:, tt, :], tpv)
                nc.sync.dma_start(out=a2a_v_in[j], in_=vloc)
            nc.gpsimd.collective_compute(
                "AllToAll", Alu.bypass, replica_groups=groups,
                ins=[a2a_v_in[:, :, :, :]], outs=[a2a_v_out[:, :, :, :]])

        # weight prefetch for later stages rides the scalar HWDGE ring now
        # (pools opened here reuse the space freed by the QKV stage)
        bigp = top.enter_context(tc.tile_pool(name="bigp", bufs=2))
        wpp = top.enter_context(tc.tile_pool(name="wpp", bufs=1))
        wproj_sb = wpp.tile([P, NG, C], bf16)
        nc.scalar.dma_start(out=wproj_sb, in_=wproj)
        w1a_sb = bigp.tile([P, NG, FF // 2], bf16, tag="big", name="w1a")
        w1b_sb = bigp.tile([P, NG, FF // 2], bf16, tag="big", name="w1b")
        nc.scalar.dma_start(out=w1a_sb, in_=w1[:, :, 0:FF // 2])
        nc.scalar.dma_start(out=w1b_sb, in_=w1[:, :, FF // 2:FF])

        # assemble Q^T/K^T/V from the A2A outputs
        nc.vector.memset(Vsb[:, :, 64:65], 1.0)
        nc.vector.memset(Vsb[:, :, 130:131], 1.0)
        for r in range(NCORE):
            nc.sync.dma_start(out=kT[:, r, :], in_=a2a_k_out[r])
            nc.sync.dma_start(out=qT[:, r, :], in_=a2a_q_out[r])
            for hp in range(LH):
                nc.sync.dma_start(
                    out=Vsb[:, NTT * r:NTT * (r + 1), 66 * hp:66 * hp + 64],
                    in_=a2a_v_out[r, :, :, 64 * hp:64 * hp + 64])

        # ------------- Stage C: attention (head-parallel) -------------------
        with ExitStack() as sc:
            atp = sc.enter_context(tc.tile_pool(name="atp", bufs=4))
            ate = sc.enter_context(tc.tile_pool(name="ate", bufs=2))
            for hp in range(LH):
                hb = 64 * hp
                for b in range(B):
                    kt0 = 16 * b  # first global 128-key-tile of batch b
                    for jq in range(4):
                        rq = 4 * b + jq  # dest core owning this query tile
                        nk = 4 * (jq + 1)
                        psPV = ps.tile([65, 512], f32, tag="bank")
                        for ike in range(0, nk, 2):
                            psS2 = ps2.tile([P, 1024], f32, tag="bank2")
                            for dd in range(2):
                                ik = ike + dd
                                rk, ck = (kt0 + ik) // 4, (kt0 + ik) % 4
                                nc.tensor.matmul(
                                    psS2[:, 512 * dd:512 * (dd + 1)],
                                    kT[hb:hb + 64, rk,
                                       P * ck:P * (ck + 1)],
                                    qT[hb:hb + 64, rq, :],
                                    start=True, stop=True)
                            pt = atp.tile([P, 1024], bf16, tag="pt")
                            nc.scalar.activation(pt, psS2, Act.Exp)
                            for dd in range(2):
                                ik = ike + dd
                                ph = pt[:, 512 * dd:512 * (dd + 1)]
                                if 512 * jq - P * ik < P:  # diagonal: mask
                                    nc.vector.tensor_mul(
                                        ph, ph, masks[ik - 4 * jq])
                                nc.tensor.matmul(
                                    psPV, Vsb[:, kt0 + ik,
                                              66 * hp:66 * hp + 65],
                                    ph, start=(ik == 0), stop=(ik == nk - 1))
                        rs = ate.tile([1, 512], f32, tag="rs")
                        nc.vector.tensor_copy(rs, psPV[64:65, :])
                        rec_f = ate.tile([1, 512], f32, tag="rec_f")
                        nc.vector.reciprocal_approx_fast(rec_f, rs)
                        rec = ate.tile([1, 512], bf16, tag="rec")
                        nc.vector.tensor_copy(rec, rec_f)
                        psBC = ps.tile([64, 512], f32, tag="bank")
                        nc.tensor.matmul(psBC, ones1, rec,
                                         start=True, stop=True)
                        bc = ate.tile([64, 512], f32, tag="bc")
                        nc.vector.tensor_copy(bc, psBC)
                        att = ate.tile([64, 512], bf16, tag="attout")
                        nc.vector.tensor_mul(att, psPV[0:64, :], bc)
                        nc.sync.dma_start(out=att_in[hp][rq], in_=att)
                nc.gpsimd.collective_compute(
                    "AllToAll", Alu.bypass, replica_groups=groups,
                    ins=[att_in[hp][:, :, :]], outs=[att_out[hp][:, :, :]])

        # ------------- Stage D: proj + residual ----------------------------
        with ExitStack() as sd:
            prp = sd.enter_context(tc.tile_pool(name="prp", bufs=8))
            ats = []
            for g in range(NG):
                at = prp.tile([P, CHUNK], bf16, tag="at", name=f"at{g}")
                nc.sync.dma_start(out=at[0:64, :], in_=att_out[0][g])
                nc.sync.dma_start(out=at[64:128, :], in_=att_out[1][g])
                ats.append(at)
            for n in range(2):
                for jp in range(2):  # token-tile pairs; 2 live accumulators
                    psj = [ps2.tile([P, 512], f32, tag="bank2",
                                    name=f"psj{n}_{jp}_{jj}")
                           for jj in range(2)]
                    for g in range(NG):
                        for jj in range(2):
                            jt = 2 * jp + jj
                            nc.tensor.matmul(
                                psj[jj], ats[g][:, P * jt:P * (jt + 1)],
                                wproj_sb[:, g, 512 * n:512 * (n + 1)],
                                start=(g == 0), stop=(g == NG - 1))
                    for jj in range(2):
                        jt = 2 * jp + jj
                        nc.vector.tensor_add(
                            xmid_sb[:, jt, 512 * n:512 * (n + 1)], psj[jj],
                            xc_sb[:, jt, 512 * n:512 * (n + 1)])

        # ------------- Stage E: LN2 + FFN + residual ------------------------
        with ExitStack() as se:
            ffp = se.enter_context(tc.tile_pool(name="ffp", bufs=1))
            lnp2 = se.enter_context(tc.tile_pool(name="lnp2", bufs=3))
            outp = se.enter_context(tc.tile_pool(name="outp", bufs=3))

            h2T = ffp.tile([P, NG, CHUNK], bf16)
            ff1T = ffp.tile([P, NF, CHUNK], bf16)

            for jt in range(NTT):
                hn2 = layernorm_tile(lnp2, xmid_sb[:, jt, :], bf16)
                for g in range(NG):
                    tp = ps.tile([P, P], bf16, tag="bank")
                    nc.tensor.transpose(tp, hn2[:, P * g:P * (g + 1)], ident)
                    nc.vector.tensor_copy(
                        h2T[:, g, P * jt:P * (jt + 1)], tp)

            # FFN1: f-slices 0..15 read w1a, 16..31 read w1b
            for f in range(NF):
                wsrc = w1a_sb if f < NF // 2 else w1b_sb
                fo = f if f < NF // 2 else f - NF // 2
                psF = ps2.tile([P, CHUNK], f32, tag="bank2")
                for g in range(NG):
                    nc.tensor.matmul(psF, wsrc[:, g, P * fo:P * (fo + 1)],
                                     h2T[:, g, :],
                                     start=(g == 0), stop=(g == NG - 1))
                nc.scalar.activation(ff1T[:, f, :], psF, Act.Relu,
                                     bias=bff1_sb[:, f:f + 1])

            # w2 prefetch recycles the w1 slots (WAR dep on last w1 reader);
            # sync ring so the waits don't block scalar-engine relu work
            w2a_sb = bigp.tile([P, NF, 512], bf16, tag="big", name="w2a")
            w2b_sb = bigp.tile([P, NF, 512], bf16, tag="big", name="w2b")
            nc.sync.dma_start(out=w2a_sb, in_=w2a)
            nc.sync.dma_start(out=w2b_sb, in_=w2b)

            for n in range(2):
                w2n = w2a_sb if n == 0 else w2b_sb
                psj = [ps.tile([P, 512], f32, tag="bank", name=f"psk{n}_{jt}")
                       for jt in range(NTT)]
                for q in range(NF):
                    for jt in range(NTT):
                        nc.tensor.matmul(
                            psj[jt], ff1T[:, q, P * jt:P * (jt + 1)],
                            w2n[:, q, :], start=(q == 0), stop=(q == NF - 1))
                for jt in range(NTT):
                    ot = outp.tile([P, 512], f32, tag="outt")
                    nc.vector.tensor_add(ot, psj[jt],
                                         xmid_sb[:, jt, 512 * n:512 * (n + 1)])
                    nc.sync.dma_start(
                        out=out[P * jt:P * (jt + 1), 512 * n:512 * (n + 1)],
                        in_=ot)

    nc.compile()
    return nc


def _prepare_inputs(x, Wq, Wk, Wv, p, Wproj, W1, W2,
                    ln1_w, ln1_b, ln2_w, ln2_b):
    import ml_dtypes
    f = np.float32
    bf = ml_dtypes.bfloat16
    x = np.asarray(x, f).reshape(TOK, C)
    Wq, Wk, Wv = (np.asarray(a, f) for a in (Wq, Wk, Wv))
    p = np.asarray(p, f)
    Wproj = np.asarray(Wproj, f)
    W1, W2 = np.asarray(W1, f), np.asarray(W2, f)
    ln1_w, ln1_b = np.asarray(ln1_w, f), np.asarray(ln1_b, f)
    ln2_w, ln2_b = np.asarray(ln2_w, f), np.asarray(ln2_b, f)

    s = (p.astype(np.float64) ** -0.5).astype(f)

    def relay(w):  # [C, M] -> [128, NG, M] partition-major contiguous
        m = w.shape[1]
        return np.ascontiguousarray(
            w.reshape(NG, P, m).transpose(1, 0, 2).astype(bf))

    # fold LN1 scale + per-head attention scale into the projections;
    # columns in natural head order (dest core j = heads 2j, 2j+1)
    wq_full = np.concatenate(
        [ln1_w[:, None] * Wq[h] * s[h] for h in range(H)], axis=1)
    wk_full = np.concatenate(
        [ln1_w[:, None] * Wk[h] for h in range(H)], axis=1)
    wv_full = np.concatenate(
        [ln1_w[:, None] * Wv[h] for h in range(H)], axis=1)
    bq_full = np.concatenate(
        [s[h] * (ln1_b @ Wq[h]) for h in range(H)])     # K bias: dropped
    bv_full = np.concatenate([ln1_b @ Wv[h] for h in range(H)])

    w1_f = ln2_w[:, None] * W1
    bff1 = np.ascontiguousarray((ln2_b @ W1).reshape(NF, P).T.astype(f))

    common = {
        "wq": relay(wq_full),
        "wk": relay(wk_full),
        "wv": relay(wv_full),
        "bq": np.ascontiguousarray(bq_full.reshape(NCORE, P).T.astype(f)),
        "bv": np.ascontiguousarray(bv_full.reshape(NCORE, P).T.astype(f)),
        "wproj": relay(Wproj),
        "w1": relay(w1_f),
        "bff1": bff1,
        "w2a": np.ascontiguousarray(
            W2[:, 0:512].reshape(NF, P, 512).transpose(1, 0, 2).astype(bf)),
        "w2b": np.ascontiguousarray(
            W2[:, 512:C].reshape(NF, P, 512).transpose(1, 0, 2).astype(bf)),
    }
    in_maps = []
    for c in range(NCORE):
        m = dict(common)
        m["xc"] = np.ascontiguousarray(x[CHUNK * c:CHUNK * (c + 1)])
        in_maps.append(m)
    return in_maps


def kernel(**inputs):
    global _BUILT
    from concourse.bass_utils import run_bass_kernel_spmd

    if _BUILT is None:
        _BUILT = _build()
    in_maps = _prepare_inputs(**inputs)
    trace = bool(int(os.environ.get("BASSK_TRACE", "0")))
    res = run_bass_kernel_spmd(_BUILT, in_maps, list(range(NCORE)),
                               trace=trace)
    if trace:
        kernel.last_exec_time_ns = res.exec_time_ns
        kernel.last_res = res
    out = np.concatenate([res.results[c]["out"] for c in range(NCORE)], axis=0)
    return out.reshape(B, T, C).astype(np.float32)
